# revision 30
# baseline (speedup 1.0000x reference)
"""GRU decoder kernel for Trainium2 — v6.

Canonical-H variant: H stays in one [128, HB] tile (full-width DVE ops),
x contributions ride one-hot matmuls on the SAME diagonal quadrants as
the h matmuls ((0,0) for u, (64,64) for v) so every PSUM accumulation
group is single-quadrant (HW-safe).  On top of that:
  - 63-step blocks with a ones-row: gate biases (b_r, b_z, b_ih_n) ride
    the one-hot matmuls -> unbiased activations.
  - r|z share one PSUM bank -> ONE sigmoid per group for both gates.
  - n-gate DVE add moved onto the PE (identity matmul accumulate).
  - optional dummy matmuls into a spare PSUM bank at the recurrence
    stall points, to keep the PE HAM clock-gate at K=8/8 (GRU_DUM=N
    columns per dummy; 0 disables).
  - static block unroll (no For_i).
"""

import os
import sys

sys.path.insert(0, "/opt/trn_rl_repo")

import numpy as np
from contextlib import ExitStack

HIDDEN = 64
OUT = 256
B = 8192
T = int(os.environ.get("GRU_T", 1024))
NCORES = 8
BC = B // NCORES          # 1024 batch per core
HB = BC // 2              # 512 batch per partition-half
SPB = 63                  # steps per block (row 63 = bias/ones row)
NBLK = (T + SPB - 1) // SPB
NGROUP = int(os.environ.get("GRU_NGROUP", 2))
DUM = int(os.environ.get("GRU_DUM", "128"))   # dummy-MM width (0 = off)
_CACHE = {}


def _steps(b):
    return min(SPB, T - b * SPB)


def _build():
    import concourse.bass as bass
    import concourse.tile as tile
    from concourse import bacc, mybir

    f16 = mybir.dt.float16
    f32 = mybir.dt.float32
    AF = mybir.ActivationFunctionType
    OP = mybir.AluOpType

    nc = bacc.Bacc("TRN2", target_bir_lowering=False, debug=False,
                   num_devices=NCORES)

    d_xu = nc.dram_tensor("xu", [128, NBLK, HB], f16, kind="ExternalInput").ap()
    d_wr = nc.dram_tensor("wr", [128, 64], f16, kind="ExternalInput").ap()
    d_wz = nc.dram_tensor("wz", [128, 64], f16, kind="ExternalInput").ap()
    d_wn = nc.dram_tensor("wn", [128, 64], f16, kind="ExternalInput").ap()
    d_ohr = nc.dram_tensor("ohr", [128, SPB, 64], f16, kind="ExternalInput").ap()
    d_ohz = nc.dram_tensor("ohz", [128, SPB, 64], f16, kind="ExternalInput").ap()
    d_ohn = nc.dram_tensor("ohn", [128, SPB, 64], f16, kind="ExternalInput").ap()
    d_eye = nc.dram_tensor("eye", [128, 64], f16, kind="ExternalInput").ap()
    d_bnh = nc.dram_tensor("bnh", [128, 1], f32, kind="ExternalInput").ap()
    d_fcw = nc.dram_tensor("fcw", [128, OUT], f16, kind="ExternalInput").ap()
    d_fcb = nc.dram_tensor("fcb", [128, 2], f32, kind="ExternalInput").ap()
    d_out = nc.dram_tensor("out", [OUT, BC], f32, kind="ExternalOutput").ap()
    d_sink = nc.dram_tensor("sink", [128, 8], f32, kind="ExternalOutput").ap()

    with tile.TileContext(nc) as tc, ExitStack() as ctx:
        singles = ctx.enter_context(tc.tile_pool(name="singles", bufs=1))
        work = ctx.enter_context(tc.tile_pool(name="work", bufs=4))
        psum = ctx.enter_context(tc.tile_pool(name="psum", bufs=1, space="PSUM"))

        XU = singles.tile([128, NBLK, HB], f16)
        WR = singles.tile([128, 64], f16)
        WZ = singles.tile([128, 64], f16)
        WN = singles.tile([128, 64], f16)
        OHR = singles.tile([128, SPB, 64], f16)
        OHZ = singles.tile([128, SPB, 64], f16)
        OHN = singles.tile([128, SPB, 64], f16)
        EYE = singles.tile([128, 64], f16)
        BNH = singles.tile([128, 1], f32)
        FCW = singles.tile([128, OUT], f16)
        FCB = singles.tile([128, 2], f32)
        H = singles.tile([128, HB], f16)

        for dst, src in ((XU, d_xu), (WR, d_wr), (WZ, d_wz), (WN, d_wn),
                         (OHR, d_ohr), (OHZ, d_ohz), (OHN, d_ohn),
                         (EYE, d_eye), (BNH, d_bnh),
                         (FCW, d_fcw), (FCB, d_fcb)):
            nc.gpsimd.dma_start(dst[:], src[:])
        nc.vector.memset(H[:], 0.0)

        HG = HB // NGROUP
        GS = [slice(g * HG, (g + 1) * HG) for g in range(NGROUP)]
        u, v = slice(0, 64), slice(64, 128)

        def dummy(xub):
            if not DUM:
                return
            D = psum.tile([128, DUM], f32, tag="dummy", name="dummy")
            nc.tensor.matmul(D[u, :], OHR[u, 0, :], xub[u, :, 0:DUM],
                             start=True, stop=True, tile_position=(0, 0),
                             skip_group_check=True)
            nc.tensor.matmul(D[v, :], OHR[v, 0, :], xub[v, :, 0:DUM],
                             start=True, stop=True, tile_position=(64, 64),
                             skip_group_check=True)

        def step(q, xub):
            bankR = psum.tile([128, HB], f32, tag="bankR", name="bankR",
                              bufs=2)
            bankZ = psum.tile([128, HB], f32, tag="bankZ", name="bankZ",
                              bufs=2)
            bankN = psum.tile([128, HB], f32, tag="bankN", name="bankN")
            bankX = psum.tile([128, HB], f32, tag="bankX", name="bankX")
            rsl, zsl = slice(0, HG), slice(HG, 2 * HG)

            # x one-hot matmuls first: no H dependency, one N=512 shot
            # covering both groups.  They OPEN the accumulation regions
            # (start=True); each group's h-matmul CLOSES its own region
            # (stop=True), so readiness keys on the per-group h-matmuls.
            # Running x early also fills the recurrence-wait PE bubble.
            nc.tensor.matmul(bankR[u, :], OHR[u, q, :], xub[u, :, :],
                             start=True, stop=False, tile_position=(0, 0),
                             skip_group_check=True)
            nc.tensor.matmul(bankR[v, :], OHR[v, q, :], xub[v, :, :],
                             start=True, stop=False, tile_position=(64, 64),
                             skip_group_check=True)
            nc.tensor.matmul(bankZ[u, :], OHZ[u, q, :], xub[u, :, :],
                             start=True, stop=False, tile_position=(0, 0),
                             skip_group_check=True)
            nc.tensor.matmul(bankZ[v, :], OHZ[v, q, :], xub[v, :, :],
                             start=True, stop=False, tile_position=(64, 64),
                             skip_group_check=True)
            for g in range(NGROUP):
                # u and v chains alternate so weight loads for one
                # row-group overlap the other row-group's matmuls
                nc.tensor.matmul(bankR[u, GS[g]], WR[u, :], H[u, GS[g]],
                                 start=False, stop=True, tile_position=(0, 0),
                                 skip_group_check=True)
                nc.tensor.matmul(bankR[v, GS[g]], WR[v, :], H[v, GS[g]],
                                 start=False, stop=True,
                                 tile_position=(64, 64),
                                 skip_group_check=True)
                nc.tensor.matmul(bankZ[u, GS[g]], WZ[u, :], H[u, GS[g]],
                                 start=False, stop=True, tile_position=(0, 0),
                                 skip_group_check=True)
                nc.tensor.matmul(bankZ[v, GS[g]], WZ[v, :], H[v, GS[g]],
                                 start=False, stop=True,
                                 tile_position=(64, 64),
                                 skip_group_check=True)
                nc.tensor.matmul(bankN[u, GS[g]], WN[u, :], H[u, GS[g]],
                                 start=True, stop=True, tile_position=(0, 0))
                nc.tensor.matmul(bankN[v, GS[g]], WN[v, :], H[v, GS[g]],
                                 start=True, stop=True,
                                 tile_position=(64, 64))
            # xn last among the gate matmuls: its WAR wait (previous
            # step's tanh reads of bankX) overlaps the h-matmuls above
            nc.tensor.matmul(bankX[u, :], OHN[u, q, :], xub[u, :, :],
                             start=True, stop=False, tile_position=(0, 0),
                             skip_group_check=True)
            nc.tensor.matmul(bankX[v, :], OHN[v, q, :], xub[v, :, :],
                             start=True, stop=False, tile_position=(64, 64),
                             skip_group_check=True)

            SRZ = [work.tile([128, 2 * HG], f16, tag=f"SRZ{g}",
                             name=f"SRZ{g}") for g in range(NGROUP)]
            T1 = [work.tile([128, HG], f16, tag=f"T1{g}", name=f"T1{g}")
                  for g in range(NGROUP)]
            NN = [work.tile([128, HG], f16, tag=f"NN{g}", name=f"NN{g}")
                  for g in range(NGROUP)]
            U = [work.tile([128, HG], f16, tag=f"U{g}", name=f"U{g}")
                 for g in range(NGROUP)]
            V = [work.tile([128, HG], f16, tag=f"V{g}", name=f"V{g}")
                 for g in range(NGROUP)]

            for g in range(NGROUP):
                # r first: it gates the STT -> EYE -> tanh chain; z is
                # only needed at the tail of the step (slack-rich)
                nc.scalar.activation(SRZ[g][:, rsl], bankR[:, GS[g]],
                                     AF.Sigmoid)
            for g in range(NGROUP):
                nc.scalar.activation(SRZ[g][:, zsl], bankZ[:, GS[g]],
                                     AF.Sigmoid)
            for g in range(NGROUP):
                nc.vector.scalar_tensor_tensor(T1[g][:], bankN[:, GS[g]],
                                               BNH[:], SRZ[g][:, rsl],
                                               op0=OP.add, op1=OP.mult)
            # PE bubble: EYE waits on the DVE; dummies keep the array hot
            dummy(xub)
            for g in range(NGROUP):
                nc.tensor.matmul(bankX[u, GS[g]], EYE[u, :], T1[g][u, :],
                                 start=False, stop=True, tile_position=(0, 0),
                                 skip_group_check=True)
                nc.tensor.matmul(bankX[v, GS[g]], EYE[v, :], T1[g][v, :],
                                 start=False, stop=True,
                                 tile_position=(64, 64),
                                 skip_group_check=True)
            for g in range(NGROUP):
                nc.scalar.activation(NN[g][:], bankX[:, GS[g]], AF.Tanh)
            for g in range(NGROUP):
                nc.vector.tensor_sub(U[g][:], H[:, GS[g]], NN[g][:])
                nc.vector.tensor_mul(V[g][:], SRZ[g][:, zsl], U[g][:])
                nc.vector.tensor_add(H[:, GS[g]], NN[g][:], V[g][:])

        for b in range(NBLK):
            xub = XU[:, slice(b, b + 1), :]
            for q in range(_steps(b)):
                step(q, xub)

        # flush the dummy bank so it has a reader (avoid dead-code risk)
        if DUM:
            D = psum.tile([128, DUM], f32, tag="dummy", name="dummyf")
            nc.tensor.matmul(D[u, :], OHR[u, 0, :], XU[u, 0:1, 0:DUM],
                             start=True, stop=True, tile_position=(0, 0),
                             skip_group_check=True)
            nc.tensor.matmul(D[v, :], OHR[v, 0, :], XU[v, 0:1, 0:DUM],
                             start=True, stop=True, tile_position=(64, 64),
                             skip_group_check=True)
            Ds = work.tile([128, 8], f32, tag="Z8")
            nc.scalar.copy(Ds[:], D[:, 0:8])
            nc.gpsimd.dma_start(d_sink[:, 0:8], Ds[:])
        else:
            Z8 = work.tile([128, 8], f32, tag="Z8")
            nc.vector.memset(Z8[:], 0.0)
            nc.gpsimd.dma_start(d_sink[:, 0:8], Z8[:])

        # Final FC: out[o, b] = sum_k fc_w[o, k] h[b, k] + fc_b[o]
        for oh in range(2):
            osl = slice(oh * 128, (oh + 1) * 128)
            fc_u = psum.tile([128, HB], f32, tag="bankN")
            fc_v = psum.tile([128, HB], f32, tag="bankX")
            nc.tensor.matmul(fc_u[:], FCW[0:64, osl], H[0:64, :],
                             start=True, stop=True, tile_position=(0, 0))
            nc.tensor.matmul(fc_v[:], FCW[64:128, osl], H[64:128, :],
                             start=True, stop=True, tile_position=(64, 0))
            Ou = work.tile([128, HB], f32, tag="Ou")
            Ov = work.tile([128, HB], f32, tag="Ov")
            nc.scalar.activation(Ou[:], fc_u[:], AF.Identity,
                                 bias=FCB[:, oh:oh + 1])
            nc.scalar.activation(Ov[:], fc_v[:], AF.Identity,
                                 bias=FCB[:, oh:oh + 1])
            nc.gpsimd.dma_start(d_out[osl, 0:HB], Ou[:])
            nc.gpsimd.dma_start(d_out[osl, HB:BC], Ov[:])

    nc.compile()
    return nc


def _host_inputs(x, w_ih, w_hh, b_ih, b_hh, fc_w, fc_b):
    f16 = np.float16
    f32 = np.float32
    x = np.asarray(x, f32)
    w_ih = np.asarray(w_ih, f32)
    w_hh = np.asarray(w_hh, f32)
    b_ih = np.asarray(b_ih, f32)
    b_hh = np.asarray(b_hh, f32)
    fc_w = np.asarray(fc_w, f32)
    fc_b = np.asarray(fc_b, f32)

    eye = np.eye(SPB, dtype=f32)

    def ohb(seg, bias):
        """[128, SPB, 64]: one-hot x w_ih rows 0-62, bias row 63; both halves."""
        w = w_ih[seg, 0]
        oh = np.einsum("pq,m->pqm", eye, w)           # [SPB, SPB, 64]
        brow = np.broadcast_to(bias, (1, SPB, 64))
        half = np.concatenate([oh, brow], 0)          # [64, SPB, 64]
        return np.concatenate([half, half], 0).astype(f16)

    def wstack(seg):
        t = w_hh[seg, :].T
        return np.vstack([t, t]).astype(f16)

    rs, zs, ns = slice(0, 64), slice(64, 128), slice(128, 192)
    shared = {
        "wr": wstack(rs),
        "wz": wstack(zs),
        "wn": wstack(ns),
        "ohr": ohb(rs, b_ih[0:64] + b_hh[0:64]),
        "ohz": ohb(zs, b_ih[64:128] + b_hh[64:128]),
        "ohn": ohb(ns, b_ih[128:192]),
        "eye": np.vstack([np.eye(64), np.eye(64)]).astype(f16),
        "bnh": np.tile(b_hh[128:192].reshape(-1, 1), (2, 1)).astype(f32),
        "fcw": np.vstack([fc_w.T, fc_w.T]).astype(f16),
        "fcb": np.stack([fc_b[0:128], fc_b[128:256]], 1).astype(f32),
    }

    in_maps = []
    for c in range(NCORES):
        xs = x[c * BC:(c + 1) * BC, :T, 0]            # [BC, T]
        xT = np.ascontiguousarray(xs.T)               # [T, BC]
        blocks = []
        for b in range(NBLK):
            nb = _steps(b)
            blk = np.concatenate(
                [xT[b * SPB:b * SPB + nb],
                 np.zeros((SPB - nb, BC), f32),
                 np.ones((1, BC), f32)], 0)           # [64, BC]
            blocks.append(blk)
        Xb = np.stack(blocks)                          # [NBLK, 64, BC]
        lo = Xb[:, :, 0:HB].transpose(1, 0, 2)         # [64, NBLK, HB] u
        hi = Xb[:, :, HB:BC].transpose(1, 0, 2)        # v
        m = dict(shared)
        m["xu"] = np.ascontiguousarray(np.concatenate([lo, hi], 0)).astype(f16)
        in_maps.append(m)
    return in_maps


def _run(in_maps, trace=False):
    from concourse import bass_utils
    if "nc" not in _CACHE:
        _CACHE["nc"] = _build()
    nc = _CACHE["nc"]
    res = bass_utils.run_bass_kernel_spmd(
        nc, in_maps, core_ids=list(range(NCORES)), trace=trace)
    return res


def kernel(**inputs):
    in_maps = _host_inputs(**inputs)
    res = _run(in_maps, trace=False)
    out = np.empty([B, OUT], np.float32)
    for c in range(NCORES):
        out[c * BC:(c + 1) * BC, :] = res.results[c]["out"].T
    return out


# revision 32
# speedup vs baseline: 1.2560x; 1.2560x over previous
"""GRU decoder kernel for Trainium2 — v6.

Canonical-H variant: H stays in one [128, HB] tile (full-width DVE ops),
x contributions ride one-hot matmuls on the SAME diagonal quadrants as
the h matmuls ((0,0) for u, (64,64) for v) so every PSUM accumulation
group is single-quadrant (HW-safe).  On top of that:
  - 63-step blocks with a ones-row: gate biases (b_r, b_z, b_ih_n) ride
    the one-hot matmuls -> unbiased activations.
  - r|z share one PSUM bank -> ONE sigmoid per group for both gates.
  - n-gate DVE add moved onto the PE (identity matmul accumulate).
  - optional dummy matmuls into a spare PSUM bank at the recurrence
    stall points, to keep the PE HAM clock-gate at K=8/8 (GRU_DUM=N
    columns per dummy; 0 disables).
  - static block unroll (no For_i).
"""

import os
import sys

sys.path.insert(0, "/opt/trn_rl_repo")

import numpy as np
from contextlib import ExitStack

HIDDEN = 64
OUT = 256
B = 8192
T = int(os.environ.get("GRU_T", 1024))
NCORES = 8
BC = B // NCORES          # 1024 batch per core
HB = BC // 2              # 512 batch per partition-half
SPB = 63                  # steps per block (row 63 = bias/ones row)
NBLK = (T + SPB - 1) // SPB
NGROUP = int(os.environ.get("GRU_NGROUP", 2))
DUM = int(os.environ.get("GRU_DUM", "96"))   # dummy-MM width (0 = off)
_CACHE = {}


def _steps(b):
    return min(SPB, T - b * SPB)


def _build():
    import concourse.bass as bass
    import concourse.tile as tile
    from concourse import bacc, mybir

    f16 = mybir.dt.float16
    f32 = mybir.dt.float32
    AF = mybir.ActivationFunctionType
    OP = mybir.AluOpType

    nc = bacc.Bacc("TRN2", target_bir_lowering=False, debug=False,
                   num_devices=NCORES)

    d_xu = nc.dram_tensor("xu", [128, NBLK, HB], f16, kind="ExternalInput").ap()
    d_wr = nc.dram_tensor("wr", [128, 64], f16, kind="ExternalInput").ap()
    d_wz = nc.dram_tensor("wz", [128, 64], f16, kind="ExternalInput").ap()
    d_wn = nc.dram_tensor("wn", [128, 64], f16, kind="ExternalInput").ap()
    d_ohr = nc.dram_tensor("ohr", [128, SPB, 64], f16, kind="ExternalInput").ap()
    d_ohz = nc.dram_tensor("ohz", [128, SPB, 64], f16, kind="ExternalInput").ap()
    d_ohn = nc.dram_tensor("ohn", [128, SPB, 64], f16, kind="ExternalInput").ap()
    d_eye = nc.dram_tensor("eye", [128, 64], f16, kind="ExternalInput").ap()
    d_bnh = nc.dram_tensor("bnh", [128, 1], f32, kind="ExternalInput").ap()
    d_fcw = nc.dram_tensor("fcw", [128, OUT], f16, kind="ExternalInput").ap()
    d_fcb = nc.dram_tensor("fcb", [128, 2], f32, kind="ExternalInput").ap()
    d_out = nc.dram_tensor("out", [OUT, BC], f32, kind="ExternalOutput").ap()
    d_sink = nc.dram_tensor("sink", [128, 8], f32, kind="ExternalOutput").ap()

    with tile.TileContext(nc) as tc, ExitStack() as ctx:
        singles = ctx.enter_context(tc.tile_pool(name="singles", bufs=1))
        work = ctx.enter_context(tc.tile_pool(name="work", bufs=4))
        psum = ctx.enter_context(tc.tile_pool(name="psum", bufs=1, space="PSUM"))

        XU = singles.tile([128, NBLK, HB], f16)
        WR = singles.tile([128, 64], f16)
        WZ = singles.tile([128, 64], f16)
        WN = singles.tile([128, 64], f16)
        OHR = singles.tile([128, SPB, 64], f16)
        OHZ = singles.tile([128, SPB, 64], f16)
        OHN = singles.tile([128, SPB, 64], f16)
        EYE = singles.tile([128, 64], f16)
        BNH = singles.tile([128, 1], f32)
        FCW = singles.tile([128, OUT], f16)
        FCB = singles.tile([128, 2], f32)
        H = singles.tile([128, HB], f16)

        for dst, src in ((XU, d_xu), (WR, d_wr), (WZ, d_wz), (WN, d_wn),
                         (OHR, d_ohr), (OHZ, d_ohz), (OHN, d_ohn),
                         (EYE, d_eye), (BNH, d_bnh),
                         (FCW, d_fcw), (FCB, d_fcb)):
            nc.gpsimd.dma_start(dst[:], src[:])
        nc.vector.memset(H[:], 0.0)

        HG = HB // NGROUP
        GS = [slice(g * HG, (g + 1) * HG) for g in range(NGROUP)]
        u, v = slice(0, 64), slice(64, 128)

        def dummy(xub):
            if not DUM:
                return
            D = psum.tile([128, DUM], f32, tag="dummy", name="dummy")
            nc.tensor.matmul(D[u, :], OHR[u, 0, :], xub[u, :, 0:DUM],
                             start=True, stop=True, tile_position=(0, 0),
                             skip_group_check=True)
            nc.tensor.matmul(D[v, :], OHR[v, 0, :], xub[v, :, 0:DUM],
                             start=True, stop=True, tile_position=(64, 64),
                             skip_group_check=True)

        def step(q, xub):
            # filler at the step head: the first h-matmuls wait on the
            # previous step's H update (DVE); keep the PE array busy
            dummy(xub)
            bankRZ = [psum.tile([128, 2 * HG], f32, tag=f"bankRZ{g}",
                                name=f"bankRZ{g}") for g in range(NGROUP)]
            bankN = [psum.tile([128, HG], f32, tag=f"bankN{g}",
                               name=f"bankN{g}") for g in range(NGROUP)]
            bankX = [psum.tile([128, HG], f32, tag=f"bankX{g}",
                               name=f"bankX{g}") for g in range(NGROUP)]
            rsl, zsl = slice(0, HG), slice(HG, 2 * HG)

            for g in range(NGROUP):
                # u and v chains alternate so weight loads for one
                # row-group overlap the other row-group's matmuls
                nc.tensor.matmul(bankRZ[g][u, rsl], WR[u, :], H[u, GS[g]],
                                 start=True, stop=False, tile_position=(0, 0))
                nc.tensor.matmul(bankRZ[g][v, rsl], WR[v, :], H[v, GS[g]],
                                 start=True, stop=False,
                                 tile_position=(64, 64))
                nc.tensor.matmul(bankRZ[g][u, rsl], OHR[u, q, :],
                                 xub[u, :, GS[g]],
                                 start=False, stop=True, tile_position=(0, 0))
                nc.tensor.matmul(bankRZ[g][v, rsl], OHR[v, q, :],
                                 xub[v, :, GS[g]],
                                 start=False, stop=True,
                                 tile_position=(64, 64))
                nc.tensor.matmul(bankRZ[g][u, zsl], WZ[u, :], H[u, GS[g]],
                                 start=True, stop=False, tile_position=(0, 0))
                nc.tensor.matmul(bankRZ[g][v, zsl], WZ[v, :], H[v, GS[g]],
                                 start=True, stop=False,
                                 tile_position=(64, 64))
                nc.tensor.matmul(bankRZ[g][u, zsl], OHZ[u, q, :],
                                 xub[u, :, GS[g]],
                                 start=False, stop=True, tile_position=(0, 0))
                nc.tensor.matmul(bankRZ[g][v, zsl], OHZ[v, q, :],
                                 xub[v, :, GS[g]],
                                 start=False, stop=True,
                                 tile_position=(64, 64))
                nc.tensor.matmul(bankN[g][u, :], WN[u, :], H[u, GS[g]],
                                 start=True, stop=True, tile_position=(0, 0))
                nc.tensor.matmul(bankN[g][v, :], WN[v, :], H[v, GS[g]],
                                 start=True, stop=True,
                                 tile_position=(64, 64))
                nc.tensor.matmul(bankX[g][u, :], OHN[u, q, :],
                                 xub[u, :, GS[g]],
                                 start=True, stop=False, tile_position=(0, 0))
                nc.tensor.matmul(bankX[g][v, :], OHN[v, q, :],
                                 xub[v, :, GS[g]],
                                 start=True, stop=False,
                                 tile_position=(64, 64))

            SRZ = [work.tile([128, 2 * HG], f16, tag=f"SRZ{g}",
                             name=f"SRZ{g}") for g in range(NGROUP)]
            T1 = [work.tile([128, HG], f16, tag=f"T1{g}", name=f"T1{g}")
                  for g in range(NGROUP)]
            NN = [work.tile([128, HG], f16, tag=f"NN{g}", name=f"NN{g}")
                  for g in range(NGROUP)]
            U = [work.tile([128, HG], f16, tag=f"U{g}", name=f"U{g}")
                 for g in range(NGROUP)]
            V = [work.tile([128, HG], f16, tag=f"V{g}", name=f"V{g}")
                 for g in range(NGROUP)]

            for g in range(NGROUP):
                # r first: it gates the STT -> EYE -> tanh chain; z is
                # only needed at the tail of the step (slack-rich)
                nc.scalar.activation(SRZ[g][:, rsl], bankRZ[g][:, rsl],
                                     AF.Sigmoid)
            for g in range(NGROUP):
                nc.scalar.activation(SRZ[g][:, zsl], bankRZ[g][:, zsl],
                                     AF.Sigmoid)
            for g in range(NGROUP):
                nc.vector.scalar_tensor_tensor(T1[g][:], bankN[g][:], BNH[:],
                                               SRZ[g][:, rsl],
                                               op0=OP.add, op1=OP.mult)
            # PE bubble: EYE waits on the DVE; dummies keep the array hot
            dummy(xub)
            for g in range(NGROUP):
                nc.tensor.matmul(bankX[g][u, :], EYE[u, :], T1[g][u, :],
                                 start=False, stop=True, tile_position=(0, 0))
                nc.tensor.matmul(bankX[g][v, :], EYE[v, :], T1[g][v, :],
                                 start=False, stop=True,
                                 tile_position=(64, 64))
            for g in range(NGROUP):
                nc.scalar.activation(NN[g][:], bankX[g][:], AF.Tanh)
            for g in range(NGROUP):
                nc.vector.tensor_sub(U[g][:], H[:, GS[g]], NN[g][:])
                nc.vector.tensor_mul(V[g][:], SRZ[g][:, zsl], U[g][:])
                nc.vector.tensor_add(H[:, GS[g]], NN[g][:], V[g][:])

        for b in range(NBLK):
            xub = XU[:, slice(b, b + 1), :]
            for q in range(_steps(b)):
                step(q, xub)

        # flush the dummy bank so it has a reader (avoid dead-code risk)
        if DUM:
            D = psum.tile([128, DUM], f32, tag="dummy", name="dummyf")
            nc.tensor.matmul(D[u, :], OHR[u, 0, :], XU[u, 0:1, 0:DUM],
                             start=True, stop=True, tile_position=(0, 0),
                             skip_group_check=True)
            nc.tensor.matmul(D[v, :], OHR[v, 0, :], XU[v, 0:1, 0:DUM],
                             start=True, stop=True, tile_position=(64, 64),
                             skip_group_check=True)
            Ds = work.tile([128, 8], f32, tag="Z8")
            nc.scalar.copy(Ds[:], D[:, 0:8])
            nc.gpsimd.dma_start(d_sink[:, 0:8], Ds[:])
        else:
            Z8 = work.tile([128, 8], f32, tag="Z8")
            nc.vector.memset(Z8[:], 0.0)
            nc.gpsimd.dma_start(d_sink[:, 0:8], Z8[:])

        # Final FC: out[o, b] = sum_k fc_w[o, k] h[b, k] + fc_b[o]
        for oh in range(2):
            osl = slice(oh * 128, (oh + 1) * 128)
            fc_u = psum.tile([128, HB], f32, tag="bankRZ0")
            fc_v = psum.tile([128, HB], f32, tag="bankRZ1")
            nc.tensor.matmul(fc_u[:], FCW[0:64, osl], H[0:64, :],
                             start=True, stop=True, tile_position=(0, 0))
            nc.tensor.matmul(fc_v[:], FCW[64:128, osl], H[64:128, :],
                             start=True, stop=True, tile_position=(64, 0))
            Ou = work.tile([128, HB], f32, tag="Ou")
            Ov = work.tile([128, HB], f32, tag="Ov")
            nc.scalar.activation(Ou[:], fc_u[:], AF.Identity,
                                 bias=FCB[:, oh:oh + 1])
            nc.scalar.activation(Ov[:], fc_v[:], AF.Identity,
                                 bias=FCB[:, oh:oh + 1])
            nc.gpsimd.dma_start(d_out[osl, 0:HB], Ou[:])
            nc.gpsimd.dma_start(d_out[osl, HB:BC], Ov[:])

    nc.compile()
    return nc


def _host_inputs(x, w_ih, w_hh, b_ih, b_hh, fc_w, fc_b):
    f16 = np.float16
    f32 = np.float32
    x = np.asarray(x, f32)
    w_ih = np.asarray(w_ih, f32)
    w_hh = np.asarray(w_hh, f32)
    b_ih = np.asarray(b_ih, f32)
    b_hh = np.asarray(b_hh, f32)
    fc_w = np.asarray(fc_w, f32)
    fc_b = np.asarray(fc_b, f32)

    eye = np.eye(SPB, dtype=f32)

    def ohb(seg, bias):
        """[128, SPB, 64]: one-hot x w_ih rows 0-62, bias row 63; both halves."""
        w = w_ih[seg, 0]
        oh = np.einsum("pq,m->pqm", eye, w)           # [SPB, SPB, 64]
        brow = np.broadcast_to(bias, (1, SPB, 64))
        half = np.concatenate([oh, brow], 0)          # [64, SPB, 64]
        return np.concatenate([half, half], 0).astype(f16)

    def wstack(seg):
        t = w_hh[seg, :].T
        return np.vstack([t, t]).astype(f16)

    rs, zs, ns = slice(0, 64), slice(64, 128), slice(128, 192)
    shared = {
        "wr": wstack(rs),
        "wz": wstack(zs),
        "wn": wstack(ns),
        "ohr": ohb(rs, b_ih[0:64] + b_hh[0:64]),
        "ohz": ohb(zs, b_ih[64:128] + b_hh[64:128]),
        "ohn": ohb(ns, b_ih[128:192]),
        "eye": np.vstack([np.eye(64), np.eye(64)]).astype(f16),
        "bnh": np.tile(b_hh[128:192].reshape(-1, 1), (2, 1)).astype(f32),
        "fcw": np.vstack([fc_w.T, fc_w.T]).astype(f16),
        "fcb": np.stack([fc_b[0:128], fc_b[128:256]], 1).astype(f32),
    }

    in_maps = []
    for c in range(NCORES):
        xs = x[c * BC:(c + 1) * BC, :T, 0]            # [BC, T]
        xT = np.ascontiguousarray(xs.T)               # [T, BC]
        blocks = []
        for b in range(NBLK):
            nb = _steps(b)
            blk = np.concatenate(
                [xT[b * SPB:b * SPB + nb],
                 np.zeros((SPB - nb, BC), f32),
                 np.ones((1, BC), f32)], 0)           # [64, BC]
            blocks.append(blk)
        Xb = np.stack(blocks)                          # [NBLK, 64, BC]
        lo = Xb[:, :, 0:HB].transpose(1, 0, 2)         # [64, NBLK, HB] u
        hi = Xb[:, :, HB:BC].transpose(1, 0, 2)        # v
        m = dict(shared)
        m["xu"] = np.ascontiguousarray(np.concatenate([lo, hi], 0)).astype(f16)
        in_maps.append(m)
    return in_maps


def _run(in_maps, trace=False):
    from concourse import bass_utils
    if "nc" not in _CACHE:
        _CACHE["nc"] = _build()
    nc = _CACHE["nc"]
    res = bass_utils.run_bass_kernel_spmd(
        nc, in_maps, core_ids=list(range(NCORES)), trace=trace)
    return res


def kernel(**inputs):
    in_maps = _host_inputs(**inputs)
    res = _run(in_maps, trace=False)
    out = np.empty([B, OUT], np.float32)
    for c in range(NCORES):
        out[c * BC:(c + 1) * BC, :] = res.results[c]["out"].T
    return out
